# revision 18
# baseline (speedup 1.0000x reference)
"""Trainium2 Bass kernel for nn_InterpretableAttention (B=8, N=4096, DIM=1024).

Math: the reference returns softmax(q @ k^T, axis=-1)[:, 0, :] -- only row 0
of the attention matrix. So per batch b:
    q0       = Wq @ x[b,0] + bq                                  [DIM]
    v        = Wk^T @ q0                                         [DIM]
    scores_m = x[b,m] . v   (+ q0.bk, a constant -> cancels in softmax)
    out[b]   = softmax(scores)                                   [N]
bk never affects the output. The N x N score matrix and the full q/k
projections are never materialized.

Sharding: data-parallel over batch, one batch per NeuronCore (B == 8 cores).
No collectives (a ReduceScatter-based tensor-parallel phase A measured ~90us
of serial latency on this stack).

Precision: everything is pre-cast to fp16 on the host. x ~ N(0,1) and the
weights are U(-1/32, 1/32), so fp16's range is ample and its 10-bit mantissa
keeps the end-to-end rel err at ~1.2e-3 (measured on the fixed seed, incl.
worst-case subnormal flushing) while halving weight HBM traffic vs f32 and
making every matmul single-pass on the PE. Per-core DMA: 8.4MB x + 4.2MB W.

Layout trick for the softmax: the 4096 scores are accumulated as a PSUM
[8, 512] tile -- m-tile t lands on PARTITION t, selected by a [128, 8]
stationary that holds the v-chunk in column t and zeros elsewhere. The
whole softmax then runs 8-lane-parallel (DVE max, ACT exp+accum, scale),
with 8-value cross-partition max/sum done by tiny PE matmuls against
host-supplied identity/ones constants. A row-shaped [1, 4096] softmax
costs ~14us in single-lane engine time; this shape costs ~4us.

Phase A streams the weights through the PE as moving operands (x0 / q0
chunk columns stationary), so no [128,128] LDWEIGHTS anywhere. The q0/v
rows are redistributed to column layout with 8 fp16 ones-vector matmuls.
bq is added via an extra K=1 matmul accumulated into the q0 PSUM chain.
"""

from contextlib import ExitStack

import numpy as np

import concourse.bass as bass  # noqa: F401
import concourse.tile as tile
from concourse import bacc, mybir
from concourse.bass_utils import run_bass_kernel_spmd

B, N, DIM = 8, 4096, 1024
P = 128          # partitions
KC = DIM // P    # 8 chunks along d (or e)
XT = 2048        # x DMA tile free size (4KB lines in fp16)
NXT = N // XT    # 2 x-tiles per chunk row
ST = 512         # scores per psum partition row
F32 = mybir.dt.float32
F16 = mybir.dt.float16

_program_cache = {}


def _build_program():
    if "nc" in _program_cache:
        return _program_cache["nc"]

    nc = bacc.Bacc(
        "TRN2",
        target_bir_lowering=False,
        debug=False,
        enable_asserts=False,
        num_devices=B,
    )
    xtb = nc.dram_tensor("xtb", [DIM, N], F16, kind="ExternalInput").ap()
    wqt = nc.dram_tensor("wqt", [DIM, DIM], F16, kind="ExternalInput").ap()
    wk = nc.dram_tensor("wk", [DIM, DIM], F16, kind="ExternalInput").ap()
    x0r = nc.dram_tensor("x0r", [P, KC], F16, kind="ExternalInput").ap()
    bqr = nc.dram_tensor("bqr", [1, DIM], F16, kind="ExternalInput").ap()
    ones1h = nc.dram_tensor("ones1h", [1, 1], F16, kind="ExternalInput").ap()
    ident8 = nc.dram_tensor("ident8", [8, 8], F32, kind="ExternalInput").ap()
    ones8c = nc.dram_tensor("ones8c", [8, 1], F32, kind="ExternalInput").ap()
    ones8r = nc.dram_tensor("ones8r", [1, 8], F32, kind="ExternalInput").ap()
    out = nc.dram_tensor("out", [1, N], F32, kind="ExternalOutput").ap()

    with tile.TileContext(nc) as tc, ExitStack() as ctx:
        singles = ctx.enter_context(tc.tile_pool(name="singles", bufs=1))
        wqpool = ctx.enter_context(tc.tile_pool(name="wqpool", bufs=8))
        wkpool = ctx.enter_context(tc.tile_pool(name="wkpool", bufs=8))
        xpool = ctx.enter_context(tc.tile_pool(name="xpool", bufs=8))
        psA = ctx.enter_context(tc.tile_pool(name="psA", bufs=1, space="PSUM"))
        psT = ctx.enter_context(tc.tile_pool(name="psT", bufs=1, space="PSUM"))
        psB = ctx.enter_context(tc.tile_pool(name="psB", bufs=1, space="PSUM"))

        queues = [nc.sync, nc.gpsimd, nc.scalar]

        # x0 first (the q-chain's stationary), weights right behind it
        x0s = singles.tile([P, KC], F16)
        nc.sync.dma_start(x0s, x0r)

        # ---------------- Phase A: q0^T = x0^T WqT + bq ----------------
        qps = psA.tile([1, DIM], F32, name="psa")
        for i in range(KC):
            wq_t = wqpool.tile([P, DIM], F16)
            queues[i % 3].dma_start(wq_t, wqt[i * P : (i + 1) * P, :])
            for h in range(2):
                nc.tensor.matmul(
                    qps[:, h * 512 : (h + 1) * 512],
                    x0s[:, i : i + 1],
                    wq_t[:, h * 512 : (h + 1) * 512],
                    start=(i == 0),
                    stop=False,
                )
        # small constants (needed only from the bias add onwards)
        bqs = singles.tile([1, DIM], F16)
        nc.gpsimd.dma_start(bqs, bqr)
        ones1 = singles.tile([1, 1], F16)
        nc.scalar.dma_start(ones1, ones1h)
        id8 = singles.tile([8, 8], F32)
        nc.sync.dma_start(id8, ident8)
        for h in range(2):  # bias via K=1 matmul folded into the accum chain
            nc.tensor.matmul(
                qps[:, h * 512 : (h + 1) * 512],
                ones1,
                bqs[:, h * 512 : (h + 1) * 512],
                start=False,
                stop=True,
            )
        q0row = singles.tile([1, DIM], F16)
        nc.vector.tensor_copy(q0row[:, 0:512], qps[:, 0:512])
        nc.scalar.copy(q0row[:, 512:1024], qps[:, 512:1024])

        # transpose q0 row -> [128, KC] fp16 columns via ones-vector matmuls
        tps = psT.tile([P, KC], F32, name="pst")
        for i in range(KC):
            nc.tensor.matmul(
                tps[:, i : i + 1],
                q0row[0:1, i * P : (i + 1) * P],
                ones1,
                start=True,
                stop=True,
            )
        q0c = singles.tile([P, KC], F16)
        nc.vector.tensor_copy(q0c, tps)

        # ---------------- Phase A2: v^T = q0^T' Wk ----------------
        vps = psA.tile([1, DIM], F32, name="psa")
        for j in range(KC):
            wk_t = wkpool.tile([P, DIM], F16)
            queues[j % 3].dma_start(wk_t, wk[j * P : (j + 1) * P, :])
            for h in range(2):
                nc.tensor.matmul(
                    vps[:, h * 512 : (h + 1) * 512],
                    q0c[:, j : j + 1],
                    wk_t[:, h * 512 : (h + 1) * 512],
                    start=(j == 0),
                    stop=(j == KC - 1),
                )
        vrow = singles.tile([1, DIM], F16)
        nc.vector.tensor_copy(vrow[:, 0:512], vps[:, 0:512])
        nc.scalar.copy(vrow[:, 512:1024], vps[:, 512:1024])

        # transpose v row -> [128, KC] fp16 columns
        tps2 = psT.tile([P, KC], F32, name="pst")
        for i in range(KC):
            nc.tensor.matmul(
                tps2[:, i : i + 1],
                vrow[0:1, i * P : (i + 1) * P],
                ones1,
                start=True,
                stop=True,
            )
        vc = singles.tile([P, KC], F16)
        nc.vector.tensor_copy(vc, tps2)

        # column-selected stationaries: vsel[:, t, k, c] = v-chunk k iff c == t,
        # so m-tile t's scores accumulate on PSUM partition t
        vsel = singles.tile([P, 8, KC, 8], F16)
        nc.vector.memset(vsel, 0.0)
        for t in range(8):
            nc.vector.tensor_copy(vsel[:, t, :, t], vc)

        # ---------------- Phase B: scores as PSUM [8, 512] ----------------
        ps8 = psB.tile([8, ST], F32)
        for bt in range(NXT):
            for k in range(KC):
                xt_t = xpool.tile([P, XT], F16)
                eng = queues[(bt * KC + k) % 3]
                eng.dma_start(
                    xt_t, xtb[k * P : (k + 1) * P, bt * XT : (bt + 1) * XT]
                )
                for h in range(4):
                    t = bt * 4 + h
                    nc.tensor.matmul(
                        ps8,
                        vsel[:, t, k, :],
                        xt_t[:, h * 512 : (h + 1) * 512],
                        start=(bt == 0 and k == 0 and h == 0),
                        stop=(bt == NXT - 1 and k == KC - 1 and h == 3),
                    )

        # ---------------- Phase C: softmax, 8-lane parallel ----------------
        # exp(s - M_p) per partition row immediately; the exp(M_p - gmax)/Z
        # correction rides on the final per-row scale.
        nsmax8 = singles.tile([8, 1], F32)
        nc.vector.tensor_reduce(
            nsmax8, ps8, axis=mybir.AxisListType.X, op=mybir.AluOpType.max, negate=True
        )
        esb8 = singles.tile([8, ST], F32)
        ssum8 = singles.tile([8, 1], F32)
        nc.scalar.activation(
            esb8,
            ps8,
            mybir.ActivationFunctionType.Exp,
            bias=nsmax8,
            scale=1.0,
            accum_out=ssum8,
        )
        # row forms of -M and S via PE transposes (overlap the big exp)
        negmrow = psT.tile([1, 8], F32, name="negmrow")
        nc.tensor.matmul(negmrow, nsmax8, id8, start=True, stop=True)
        nggmax = singles.tile([1, 1], F32)  # min(-M) == -gmax
        nc.vector.tensor_reduce(
            nggmax, negmrow, axis=mybir.AxisListType.X, op=mybir.AluOpType.min
        )
        w8row = singles.tile([1, 8], F32)  # exp(M - gmax)
        nc.scalar.activation(
            w8row, negmrow, mybir.ActivationFunctionType.Exp, bias=nggmax, scale=-1.0
        )
        srow = psT.tile([1, 8], F32, name="srow")
        nc.tensor.matmul(srow, ssum8, id8, start=True, stop=True)
        zsc = singles.tile([1, 8], F32)
        ztot = singles.tile([1, 1], F32)  # Z = sum_p S_p * w_p
        nc.vector.tensor_mul(zsc, srow, w8row)
        nc.vector.tensor_reduce(
            ztot, zsc, axis=mybir.AxisListType.X, op=mybir.AluOpType.add
        )
        rinv = singles.tile([1, 1], F32)
        nc.vector.reciprocal(rinv, ztot)
        rrow = singles.tile([1, 8], F32)  # w_p / Z
        nc.vector.tensor_scalar_mul(rrow, w8row, rinv)
        r8p = psT.tile([8, 1], F32, name="r8p")
        nc.tensor.matmul(r8p, rrow, id8[0:1, 0:1], start=True, stop=True)
        r8 = singles.tile([8, 1], F32)
        nc.vector.tensor_copy(r8, r8p)

        osb8 = singles.tile([8, ST], F32)
        nc.scalar.activation(
            osb8[:, 0:256], esb8[:, 0:256], mybir.ActivationFunctionType.Copy,
            bias=0.0, scale=r8,
        )
        nc.vector.tensor_scalar_mul(osb8[:, 256:ST], esb8[:, 256:ST], r8)
        nc.sync.dma_start(out.rearrange("u (t m) -> (u t) m", t=8), osb8)

    nc.compile()
    _program_cache["nc"] = nc
    return nc


def _make_in_maps(x, Wq, bq, Wk):
    x = np.asarray(x, dtype=np.float32)
    wq = np.asarray(Wq, np.float32)
    wk = np.asarray(Wk, np.float32)
    bq = np.asarray(bq, np.float32)
    wqt_h = np.ascontiguousarray(wq.T.astype(np.float16))
    wk_h = np.ascontiguousarray(wk.astype(np.float16))
    bq_h = np.ascontiguousarray(bq.reshape(1, DIM).astype(np.float16))
    consts = {
        "ones1h": np.ones((1, 1), np.float16),
        "ident8": np.eye(8, dtype=np.float32),
        "ones8c": np.ones((8, 1), np.float32),
        "ones8r": np.ones((1, 8), np.float32),
    }
    in_maps = []
    for b in range(B):
        in_maps.append(
            {
                "xtb": np.ascontiguousarray(x[b].T.astype(np.float16)),
                "wqt": wqt_h,
                "wk": wk_h,
                "x0r": np.ascontiguousarray(
                    x[b, 0].reshape(KC, P).T.astype(np.float16)
                ),
                "bqr": bq_h,
                **consts,
            }
        )
    return in_maps


def kernel(x, Wq, bq, Wk, bk):
    nc = _build_program()
    in_maps = _make_in_maps(x, Wq, bq, Wk)
    res = run_bass_kernel_spmd(nc, in_maps, core_ids=list(range(B)))
    outs = [np.asarray(res.results[b]["out"]).reshape(N) for b in range(B)]
    return np.stack(outs, axis=0).astype(np.float32)


# revision 19
# speedup vs baseline: 1.0596x; 1.0596x over previous
"""Trainium2 Bass kernel for nn_InterpretableAttention (B=8, N=4096, DIM=1024).

Math: the reference returns softmax(q @ k^T, axis=-1)[:, 0, :] -- only row 0
of the attention matrix. So per batch b:
    q0       = Wq @ x[b,0] + bq                                  [DIM]
    v        = Wk^T @ q0                                         [DIM]
    scores_m = x[b,m] . v   (+ q0.bk, a constant -> cancels in softmax)
    out[b]   = softmax(scores)                                   [N]
bk never affects the output. The N x N score matrix and the full q/k
projections are never materialized.

Sharding: data-parallel over batch, one batch per NeuronCore (B == 8 cores).
No collectives (a ReduceScatter-based tensor-parallel phase A measured ~90us
of serial latency on this stack).

Precision: everything is pre-cast to fp16 on the host. x ~ N(0,1) and the
weights are U(-1/32, 1/32), so fp16's range is ample and its 10-bit mantissa
keeps the end-to-end rel err at ~1.2e-3 (measured on the fixed seed, incl.
worst-case subnormal flushing) while halving weight HBM traffic vs f32 and
making every matmul single-pass on the PE. Per-core DMA: 8.4MB x + 4.2MB W.

Layout trick for the softmax: the 4096 scores are accumulated as a PSUM
[8, 512] tile -- m-tile t lands on PARTITION t, selected by a [128, 8]
stationary that holds the v-chunk in column t and zeros elsewhere. The
whole softmax then runs 8-lane-parallel (DVE max, ACT exp+accum, scale),
with 8-value cross-partition max/sum done by tiny PE matmuls against
host-supplied identity/ones constants. A row-shaped [1, 4096] softmax
costs ~14us in single-lane engine time; this shape costs ~4us.

Phase A streams the weights through the PE as moving operands (x0 / q0
chunk columns stationary), so no [128,128] LDWEIGHTS anywhere. The q0/v
rows are redistributed to column layout with 8 fp16 ones-vector matmuls.
bq is added via an extra K=1 matmul accumulated into the q0 PSUM chain.
"""

from contextlib import ExitStack

import numpy as np

import concourse.bass as bass  # noqa: F401
import concourse.tile as tile
from concourse import bacc, mybir
from concourse.bass_utils import run_bass_kernel_spmd

B, N, DIM = 8, 4096, 1024
P = 128          # partitions
KC = DIM // P    # 8 chunks along d (or e)
XT = 1024        # x DMA tile free size (2KB lines in fp16)
NXT = N // XT    # 4 x-tiles
ST = 512         # scores per psum partition row
F32 = mybir.dt.float32
F16 = mybir.dt.float16

_program_cache = {}


def _build_program():
    if "nc" in _program_cache:
        return _program_cache["nc"]

    nc = bacc.Bacc(
        "TRN2",
        target_bir_lowering=False,
        debug=False,
        enable_asserts=False,
        num_devices=B,
    )
    xtb = nc.dram_tensor("xtb", [DIM, N], F16, kind="ExternalInput").ap()
    wqt = nc.dram_tensor("wqt", [DIM, DIM], F16, kind="ExternalInput").ap()
    wk = nc.dram_tensor("wk", [DIM, DIM], F16, kind="ExternalInput").ap()
    x0r = nc.dram_tensor("x0r", [P, KC], F16, kind="ExternalInput").ap()
    bqr = nc.dram_tensor("bqr", [1, DIM], F16, kind="ExternalInput").ap()
    ones1h = nc.dram_tensor("ones1h", [1, 1], F16, kind="ExternalInput").ap()
    ident8 = nc.dram_tensor("ident8", [8, 8], F32, kind="ExternalInput").ap()
    ones8c = nc.dram_tensor("ones8c", [8, 1], F32, kind="ExternalInput").ap()
    ones8r = nc.dram_tensor("ones8r", [1, 8], F32, kind="ExternalInput").ap()
    out = nc.dram_tensor("out", [1, N], F32, kind="ExternalOutput").ap()

    with tile.TileContext(nc) as tc, ExitStack() as ctx:
        singles = ctx.enter_context(tc.tile_pool(name="singles", bufs=1))
        wqpool = ctx.enter_context(tc.tile_pool(name="wqpool", bufs=8))
        wkpool = ctx.enter_context(tc.tile_pool(name="wkpool", bufs=8))
        xpool = ctx.enter_context(tc.tile_pool(name="xpool", bufs=16))
        psA = ctx.enter_context(tc.tile_pool(name="psA", bufs=1, space="PSUM"))
        psT = ctx.enter_context(tc.tile_pool(name="psT", bufs=1, space="PSUM"))
        psB = ctx.enter_context(tc.tile_pool(name="psB", bufs=1, space="PSUM"))

        queues = [nc.sync, nc.gpsimd, nc.scalar]

        # small inputs / constants
        x0s = singles.tile([P, KC], F16)
        nc.sync.dma_start(x0s, x0r)
        bqs = singles.tile([1, DIM], F16)
        nc.gpsimd.dma_start(bqs, bqr)
        ones1 = singles.tile([1, 1], F16)
        nc.scalar.dma_start(ones1, ones1h)
        id8 = singles.tile([8, 8], F32)
        nc.sync.dma_start(id8, ident8)
        o8c = singles.tile([8, 1], F32)
        nc.gpsimd.dma_start(o8c, ones8c)
        o8r = singles.tile([1, 8], F32)
        nc.scalar.dma_start(o8r, ones8r)

        # ---------------- Phase A: q0^T = x0^T WqT + bq ----------------
        qps = psA.tile([1, DIM], F32, name="psa")
        for h in range(2):  # bias via K=1 matmul folded into the accum chain
            nc.tensor.matmul(
                qps[:, h * 512 : (h + 1) * 512],
                ones1,
                bqs[:, h * 512 : (h + 1) * 512],
                start=True,
                stop=False,
            )
        for i in range(KC):
            wq_t = wqpool.tile([P, DIM], F16)
            queues[i % 3].dma_start(wq_t, wqt[i * P : (i + 1) * P, :])
            for h in range(2):
                nc.tensor.matmul(
                    qps[:, h * 512 : (h + 1) * 512],
                    x0s[:, i : i + 1],
                    wq_t[:, h * 512 : (h + 1) * 512],
                    start=False,
                    stop=(i == KC - 1),
                )
        q0row = singles.tile([1, DIM], F16)
        nc.vector.tensor_copy(q0row[:, 0:512], qps[:, 0:512])
        nc.scalar.copy(q0row[:, 512:1024], qps[:, 512:1024])

        # transpose q0 row -> [128, KC] fp16 columns via ones-vector matmuls
        tps = psT.tile([P, KC], F32, name="pst")
        for i in range(KC):
            nc.tensor.matmul(
                tps[:, i : i + 1],
                q0row[0:1, i * P : (i + 1) * P],
                ones1,
                start=True,
                stop=True,
            )
        q0c = singles.tile([P, KC], F16)
        nc.vector.tensor_copy(q0c, tps)

        # ---------------- Phase A2: v^T = q0^T' Wk ----------------
        vps = psA.tile([1, DIM], F32, name="psa")
        for j in range(KC):
            wk_t = wkpool.tile([P, DIM], F16)
            queues[j % 3].dma_start(wk_t, wk[j * P : (j + 1) * P, :])
            for h in range(2):
                nc.tensor.matmul(
                    vps[:, h * 512 : (h + 1) * 512],
                    q0c[:, j : j + 1],
                    wk_t[:, h * 512 : (h + 1) * 512],
                    start=(j == 0),
                    stop=(j == KC - 1),
                )
        vrow = singles.tile([1, DIM], F16)
        nc.vector.tensor_copy(vrow[:, 0:512], vps[:, 0:512])
        nc.scalar.copy(vrow[:, 512:1024], vps[:, 512:1024])

        # transpose v row -> [128, KC] fp16 columns
        tps2 = psT.tile([P, KC], F32, name="pst")
        for i in range(KC):
            nc.tensor.matmul(
                tps2[:, i : i + 1],
                vrow[0:1, i * P : (i + 1) * P],
                ones1,
                start=True,
                stop=True,
            )
        vc = singles.tile([P, KC], F16)
        nc.vector.tensor_copy(vc, tps2)

        # column-selected stationaries: vsel[:, t, k, c] = v-chunk k iff c == t,
        # so m-tile t's scores accumulate on PSUM partition t
        vsel = singles.tile([P, 8, KC, 8], F16)
        nc.vector.memset(vsel, 0.0)
        for t in range(8):
            nc.vector.tensor_copy(vsel[:, t, :, t], vc)

        # ---------------- Phase B: scores as PSUM [8, 512] ----------------
        ps8 = psB.tile([8, ST], F32)
        for bt in range(NXT):
            for k in range(KC):
                xt_t = xpool.tile([P, XT], F16)
                eng = queues[(bt * KC + k) % 3]
                eng.dma_start(
                    xt_t, xtb[k * P : (k + 1) * P, bt * XT : (bt + 1) * XT]
                )
                for h in range(2):
                    t = bt * 2 + h
                    nc.tensor.matmul(
                        ps8,
                        vsel[:, t, k, :],
                        xt_t[:, h * 512 : (h + 1) * 512],
                        start=(bt == 0 and k == 0 and h == 0),
                        stop=(bt == NXT - 1 and k == KC - 1 and h == 1),
                    )

        # ---------------- Phase C: softmax, 8-lane parallel ----------------
        smax8 = singles.tile([8, 1], F32)
        nc.vector.tensor_reduce(
            smax8, ps8, axis=mybir.AxisListType.X, op=mybir.AluOpType.max
        )
        mrow = psT.tile([1, 8], F32, name="mrow")
        nc.tensor.matmul(mrow, smax8, id8, start=True, stop=True)
        negmax = singles.tile([1, 1], F32)
        nc.vector.tensor_reduce(
            negmax, mrow, axis=mybir.AxisListType.X, op=mybir.AluOpType.max, negate=True
        )
        nm8p = psT.tile([8, 1], F32, name="nm8p")
        nc.tensor.matmul(nm8p, o8r, negmax, start=True, stop=True)
        nm8 = singles.tile([8, 1], F32)
        nc.vector.tensor_copy(nm8, nm8p)

        esb8 = singles.tile([8, ST], F32)
        ssum8 = singles.tile([8, 1], F32)
        nc.scalar.activation(
            esb8,
            ps8,
            mybir.ActivationFunctionType.Exp,
            bias=nm8,
            scale=1.0,
            accum_out=ssum8,
        )
        totp = psT.tile([1, 1], F32, name="totp")
        nc.tensor.matmul(totp, ssum8, o8c, start=True, stop=True)
        rinv = singles.tile([1, 1], F32)
        nc.vector.reciprocal(rinv, totp)
        r8p = psT.tile([8, 1], F32, name="r8p")
        nc.tensor.matmul(r8p, o8r, rinv, start=True, stop=True)
        r8 = singles.tile([8, 1], F32)
        nc.vector.tensor_copy(r8, r8p)

        osb8 = singles.tile([8, ST], F32)
        nc.scalar.activation(
            osb8, esb8, mybir.ActivationFunctionType.Copy, bias=0.0, scale=r8
        )
        nc.sync.dma_start(out.rearrange("u (t m) -> (u t) m", t=8), osb8)

    nc.compile()
    _program_cache["nc"] = nc
    return nc


def _make_in_maps(x, Wq, bq, Wk):
    x = np.asarray(x, dtype=np.float32)
    wq = np.asarray(Wq, np.float32)
    wk = np.asarray(Wk, np.float32)
    bq = np.asarray(bq, np.float32)
    wqt_h = np.ascontiguousarray(wq.T.astype(np.float16))
    wk_h = np.ascontiguousarray(wk.astype(np.float16))
    bq_h = np.ascontiguousarray(bq.reshape(1, DIM).astype(np.float16))
    consts = {
        "ones1h": np.ones((1, 1), np.float16),
        "ident8": np.eye(8, dtype=np.float32),
        "ones8c": np.ones((8, 1), np.float32),
        "ones8r": np.ones((1, 8), np.float32),
    }
    in_maps = []
    for b in range(B):
        in_maps.append(
            {
                "xtb": np.ascontiguousarray(x[b].T.astype(np.float16)),
                "wqt": wqt_h,
                "wk": wk_h,
                "x0r": np.ascontiguousarray(
                    x[b, 0].reshape(KC, P).T.astype(np.float16)
                ),
                "bqr": bq_h,
                **consts,
            }
        )
    return in_maps


def kernel(x, Wq, bq, Wk, bk):
    nc = _build_program()
    in_maps = _make_in_maps(x, Wq, bq, Wk)
    res = run_bass_kernel_spmd(nc, in_maps, core_ids=list(range(B)))
    outs = [np.asarray(res.results[b]["out"]).reshape(N) for b in range(B)]
    return np.stack(outs, axis=0).astype(np.float32)


# revision 20
# speedup vs baseline: 1.1658x; 1.1002x over previous
"""Trainium2 Bass kernel for nn_InterpretableAttention (B=8, N=4096, DIM=1024).

Math: the reference returns softmax(q @ k^T, axis=-1)[:, 0, :] -- only row 0
of the attention matrix. So per batch b:
    q0       = Wq @ x[b,0] + bq                                  [DIM]
    v        = Wk^T @ q0                                         [DIM]
    scores_m = x[b,m] . v   (+ q0.bk, a constant -> cancels in softmax)
    out[b]   = softmax(scores)                                   [N]
bk never affects the output. The N x N score matrix and the full q/k
projections are never materialized.

Sharding: data-parallel over batch, one batch per NeuronCore (B == 8 cores).
No collectives (a ReduceScatter-based tensor-parallel phase A measured ~90us
of serial latency on this stack).

Precision: everything is pre-cast to fp16 on the host. x ~ N(0,1) and the
weights are U(-1/32, 1/32), so fp16's range is ample and its 10-bit mantissa
keeps the end-to-end rel err at ~1.2e-3 (measured on the fixed seed, incl.
worst-case subnormal flushing) while halving weight HBM traffic vs f32 and
making every matmul single-pass on the PE. Per-core DMA: 8.4MB x + 4.2MB W.

Layout trick for the softmax: the 4096 scores are accumulated as a PSUM
[8, 512] tile -- m-tile t lands on PARTITION t, selected by a [128, 8]
stationary that holds the v-chunk in column t and zeros elsewhere. The
whole softmax then runs 8-lane-parallel (DVE max, ACT exp+accum, scale),
with 8-value cross-partition max/sum done by tiny PE matmuls against
host-supplied identity/ones constants. A row-shaped [1, 4096] softmax
costs ~14us in single-lane engine time; this shape costs ~4us.

Phase A streams the weights through the PE as moving operands (x0 / q0
chunk columns stationary), so no [128,128] LDWEIGHTS anywhere. The q0/v
rows are redistributed to column layout with 8 fp16 ones-vector matmuls.
bq is added via an extra K=1 matmul accumulated into the q0 PSUM chain.
"""

from contextlib import ExitStack

import numpy as np

import concourse.bass as bass  # noqa: F401
import concourse.tile as tile
from concourse import bacc, mybir
from concourse.bass_utils import run_bass_kernel_spmd

B, N, DIM = 8, 4096, 1024
P = 128          # partitions
KC = DIM // P    # 8 chunks along d (or e)
XT = 1024        # x DMA tile free size (2KB lines in fp16)
NXT = N // XT    # 4 x-tiles
ST = 512         # scores per psum partition row
F32 = mybir.dt.float32
F16 = mybir.dt.float16

_program_cache = {}


def _build_program():
    if "nc" in _program_cache:
        return _program_cache["nc"]

    nc = bacc.Bacc(
        "TRN2",
        target_bir_lowering=False,
        debug=False,
        enable_asserts=False,
        num_devices=B,
    )
    xtb = nc.dram_tensor("xtb", [DIM, N], F16, kind="ExternalInput").ap()
    wqt = nc.dram_tensor("wqt", [DIM, DIM], F16, kind="ExternalInput").ap()
    wk = nc.dram_tensor("wk", [DIM, DIM], F16, kind="ExternalInput").ap()
    x0r = nc.dram_tensor("x0r", [P, KC], F16, kind="ExternalInput").ap()
    bqr = nc.dram_tensor("bqr", [1, DIM], F16, kind="ExternalInput").ap()
    ones1h = nc.dram_tensor("ones1h", [1, 1], F16, kind="ExternalInput").ap()
    ident8 = nc.dram_tensor("ident8", [8, 8], F32, kind="ExternalInput").ap()
    ones8c = nc.dram_tensor("ones8c", [8, 1], F32, kind="ExternalInput").ap()
    ones8r = nc.dram_tensor("ones8r", [1, 8], F32, kind="ExternalInput").ap()
    out = nc.dram_tensor("out", [1, N], F32, kind="ExternalOutput").ap()

    with tile.TileContext(nc) as tc, ExitStack() as ctx:
        singles = ctx.enter_context(tc.tile_pool(name="singles", bufs=1))
        wqpool = ctx.enter_context(tc.tile_pool(name="wqpool", bufs=8))
        wkpool = ctx.enter_context(tc.tile_pool(name="wkpool", bufs=8))
        xpool = ctx.enter_context(tc.tile_pool(name="xpool", bufs=16))
        psA = ctx.enter_context(tc.tile_pool(name="psA", bufs=1, space="PSUM"))
        psT = ctx.enter_context(tc.tile_pool(name="psT", bufs=1, space="PSUM"))
        psB = ctx.enter_context(tc.tile_pool(name="psB", bufs=1, space="PSUM"))

        queues = [nc.sync, nc.gpsimd, nc.scalar]

        # small inputs / constants
        x0s = singles.tile([P, KC], F16)
        nc.sync.dma_start(x0s, x0r)
        bqs = singles.tile([1, DIM], F16)
        nc.gpsimd.dma_start(bqs, bqr)
        ones1 = singles.tile([1, 1], F16)
        nc.scalar.dma_start(ones1, ones1h)
        id8 = singles.tile([8, 8], F32)
        nc.sync.dma_start(id8, ident8)
        o8c = singles.tile([8, 1], F32)
        nc.gpsimd.dma_start(o8c, ones8c)
        o8r = singles.tile([1, 8], F32)
        nc.scalar.dma_start(o8r, ones8r)

        # ---------------- Phase A: q0^T = x0^T WqT + bq ----------------
        qps = psA.tile([1, DIM], F32, name="psa")
        for h in range(2):  # bias via K=1 matmul folded into the accum chain
            nc.tensor.matmul(
                qps[:, h * 512 : (h + 1) * 512],
                ones1,
                bqs[:, h * 512 : (h + 1) * 512],
                start=True,
                stop=False,
            )
        for i in range(KC):
            wq_t = wqpool.tile([P, DIM], F16)
            queues[i % 3].dma_start(wq_t, wqt[i * P : (i + 1) * P, :])
            for h in range(2):
                nc.tensor.matmul(
                    qps[:, h * 512 : (h + 1) * 512],
                    x0s[:, i : i + 1],
                    wq_t[:, h * 512 : (h + 1) * 512],
                    start=False,
                    stop=(i == KC - 1),
                )
        q0row = singles.tile([1, DIM], F16)
        nc.vector.tensor_copy(q0row[:, 0:512], qps[:, 0:512])
        nc.scalar.copy(q0row[:, 512:1024], qps[:, 512:1024])

        # transpose q0 row -> [128, KC] fp16 columns via ones-vector matmuls
        tps = psT.tile([P, KC], F32, name="pst")
        for i in range(KC):
            nc.tensor.matmul(
                tps[:, i : i + 1],
                q0row[0:1, i * P : (i + 1) * P],
                ones1,
                start=True,
                stop=True,
            )
        q0c = singles.tile([P, KC], F16)
        nc.vector.tensor_copy(q0c, tps)

        # ---------------- Phase A2: v^T = q0^T' Wk ----------------
        vps = psA.tile([1, DIM], F32, name="psa")
        for j in range(KC):
            wk_t = wkpool.tile([P, DIM], F16)
            queues[j % 3].dma_start(wk_t, wk[j * P : (j + 1) * P, :])
            for h in range(2):
                nc.tensor.matmul(
                    vps[:, h * 512 : (h + 1) * 512],
                    q0c[:, j : j + 1],
                    wk_t[:, h * 512 : (h + 1) * 512],
                    start=(j == 0),
                    stop=(j == KC - 1),
                )
        vrow = singles.tile([1, DIM], F16)
        nc.vector.tensor_copy(vrow[:, 0:512], vps[:, 0:512])
        nc.scalar.copy(vrow[:, 512:1024], vps[:, 512:1024])

        # transpose v row -> [128, KC] fp16 columns
        tps2 = psT.tile([P, KC], F32, name="pst")
        for i in range(KC):
            nc.tensor.matmul(
                tps2[:, i : i + 1],
                vrow[0:1, i * P : (i + 1) * P],
                ones1,
                start=True,
                stop=True,
            )
        vc = singles.tile([P, KC], F16)
        nc.vector.tensor_copy(vc, tps2)

        # column-selected stationaries: vsel[:, t, k, c] = v-chunk k iff c == t,
        # so m-tile t's scores accumulate on PSUM partition t
        vsel = singles.tile([P, 8, KC, 8], F16)
        nc.vector.memset(vsel, 0.0)
        for t in range(8):
            nc.vector.tensor_copy(vsel[:, t, :, t], vc)

        # ---------------- Phase B: scores as PSUM [8, 512] ----------------
        ps8 = psB.tile([8, ST], F32)
        for bt in range(NXT):
            for k in range(KC):
                xt_t = xpool.tile([P, XT], F16)
                eng = queues[(bt * KC + k) % 3]
                eng.dma_start(
                    xt_t, xtb[k * P : (k + 1) * P, bt * XT : (bt + 1) * XT]
                )
                for h in range(2):
                    t = bt * 2 + h
                    nc.tensor.matmul(
                        ps8,
                        vsel[:, t, k, :],
                        xt_t[:, h * 512 : (h + 1) * 512],
                        start=(bt == 0 and k == 0 and h == 0),
                        stop=(bt == NXT - 1 and k == KC - 1 and h == 1),
                    )

        # ---------------- Phase C: softmax, 8-lane parallel ----------------
        # exp(s - M_p) per partition row immediately; the exp(M_p - gmax)/Z
        # correction rides on the final per-row scale.
        nsmax8 = singles.tile([8, 1], F32)
        nc.vector.tensor_reduce(
            nsmax8, ps8, axis=mybir.AxisListType.X, op=mybir.AluOpType.max, negate=True
        )
        esb8 = singles.tile([8, ST], F32)
        ssum8 = singles.tile([8, 1], F32)
        nc.scalar.activation(
            esb8,
            ps8,
            mybir.ActivationFunctionType.Exp,
            bias=nsmax8,
            scale=1.0,
            accum_out=ssum8,
        )
        # row forms of -M and S via PE transposes (overlap the big exp)
        negmrow = psT.tile([1, 8], F32, name="negmrow")
        nc.tensor.matmul(negmrow, nsmax8, id8, start=True, stop=True)
        nggmax = singles.tile([1, 1], F32)  # min(-M) == -gmax
        nc.vector.tensor_reduce(
            nggmax, negmrow, axis=mybir.AxisListType.X, op=mybir.AluOpType.min
        )
        w8row = singles.tile([1, 8], F32)  # exp(M - gmax)
        nc.scalar.activation(
            w8row, negmrow, mybir.ActivationFunctionType.Exp, bias=nggmax, scale=-1.0
        )
        srow = psT.tile([1, 8], F32, name="srow")
        nc.tensor.matmul(srow, ssum8, id8, start=True, stop=True)
        zsc = singles.tile([1, 8], F32)
        ztot = singles.tile([1, 1], F32)  # Z = sum_p S_p * w_p
        nc.vector.tensor_mul(zsc, srow, w8row)
        nc.vector.tensor_reduce(
            ztot, zsc, axis=mybir.AxisListType.X, op=mybir.AluOpType.add
        )
        rinv = singles.tile([1, 1], F32)
        nc.vector.reciprocal(rinv, ztot)
        rrow = singles.tile([1, 8], F32)  # w_p / Z
        nc.vector.tensor_scalar_mul(rrow, w8row, rinv)
        r8p = psT.tile([8, 1], F32, name="r8p")
        nc.tensor.matmul(r8p, rrow, id8[0:1, 0:1], start=True, stop=True)
        r8 = singles.tile([8, 1], F32)
        nc.vector.tensor_copy(r8, r8p)

        osb8 = singles.tile([8, ST], F32)
        nc.scalar.activation(
            osb8[:, 0:256], esb8[:, 0:256], mybir.ActivationFunctionType.Copy,
            bias=0.0, scale=r8,
        )
        nc.vector.tensor_scalar_mul(osb8[:, 256:ST], esb8[:, 256:ST], r8)
        nc.sync.dma_start(out.rearrange("u (t m) -> (u t) m", t=8), osb8)

    nc.compile()
    _program_cache["nc"] = nc
    return nc


def _make_in_maps(x, Wq, bq, Wk):
    x = np.asarray(x, dtype=np.float32)
    wq = np.asarray(Wq, np.float32)
    wk = np.asarray(Wk, np.float32)
    bq = np.asarray(bq, np.float32)
    wqt_h = np.ascontiguousarray(wq.T.astype(np.float16))
    wk_h = np.ascontiguousarray(wk.astype(np.float16))
    bq_h = np.ascontiguousarray(bq.reshape(1, DIM).astype(np.float16))
    consts = {
        "ones1h": np.ones((1, 1), np.float16),
        "ident8": np.eye(8, dtype=np.float32),
        "ones8c": np.ones((8, 1), np.float32),
        "ones8r": np.ones((1, 8), np.float32),
    }
    in_maps = []
    for b in range(B):
        in_maps.append(
            {
                "xtb": np.ascontiguousarray(x[b].T.astype(np.float16)),
                "wqt": wqt_h,
                "wk": wk_h,
                "x0r": np.ascontiguousarray(
                    x[b, 0].reshape(KC, P).T.astype(np.float16)
                ),
                "bqr": bq_h,
                **consts,
            }
        )
    return in_maps


def kernel(x, Wq, bq, Wk, bk):
    nc = _build_program()
    in_maps = _make_in_maps(x, Wq, bq, Wk)
    res = run_bass_kernel_spmd(nc, in_maps, core_ids=list(range(B)))
    outs = [np.asarray(res.results[b]["out"]).reshape(N) for b in range(B)]
    return np.stack(outs, axis=0).astype(np.float32)
